# revision 11
# baseline (speedup 1.0000x reference)
"""Trainium2 Bass kernel for CellPathwayPoolingAggregator (segment mean).

out[b, p] = (1/segment_sizes[p]) * sum_{k: segment_ids[k]==p} x[b, flat_indices[k]]

Strategy (8 cores, data-parallel over batch):
  - Host: transpose x -> xT (G, B) fp16; shard batch 8 x 256.
  - Device (per core): gpsimd.dma_gather pulls the K indexed gene-rows
    (256 fp16 = 512B each) from DRAM into SBUF laid out [k%128, k//128, b].
    Each 128-row K-tile spans a narrow window of (sorted) segment ids, so a
    PE matmul with a tiny banded one-hot matrix S (128 x W) accumulates
    per-segment sums for that tile directly into PSUM (batch x pathway).
  - DVE multiplies by 1/segment_sizes, DMA stores f32 output.
"""

import sys

import numpy as np

_TRN_REPO = "/opt/trn_rl_repo"
if _TRN_REPO not in sys.path:
    sys.path.insert(0, _TRN_REPO)

import concourse.bass as bass
import concourse.mybir as mybir
import concourse.tile as tile
from concourse import bacc
from concourse.bass_utils import run_bass_kernel_spmd

B, G, P = 2048, 10000, 1000
NCORES = 8
BC = B // NCORES  # per-core batch (256)
BANK = 512        # psum bank width in f32 elements
NBANKS = (P + BANK - 1) // BANK  # 2
CH = 8            # gather-chunk size in 128-index tiles (<=1024 idxs per dma_gather)
NQ = 4            # SWDGE queues, round-robin per chunk


def _build_schedule(flat_indices, segment_ids):
    """Sort/pad the index list and derive per-tile segment windows.

    Returns (idx_sb, S_sb, win_lo, win_n, Wmax, T, tiles_per_bank).
    Padding rules: entries are grouped by psum bank (512 pathways each) and
    each bank's entry list is padded to a multiple of 128 so no 128-row
    K-tile spans two banks. Pad entries gather gene 0 with an all-zero S row.
    Windows are stretched to cover every pathway column in the bank so PSUM
    has no unwritten (garbage) elements.
    """
    seg = np.asarray(segment_ids, dtype=np.int64)
    idx = np.asarray(flat_indices, dtype=np.int64)
    order = np.argsort(seg, kind="stable")
    seg = seg[order]
    idx = idx[order]

    parts = []
    for b in range(NBANKS):
        lo = np.searchsorted(seg, b * BANK, side="left")
        hi = np.searchsorted(seg, min((b + 1) * BANK, P), side="left")
        pi, ps = idx[lo:hi], seg[lo:hi]
        pad = (-len(pi)) % 128
        if len(pi) + pad == 0:
            pad = 128  # keep >=1 tile per bank so every psum element is written
        pi = np.concatenate([pi, np.zeros(pad, np.int64)])
        ps = np.concatenate([ps, np.full(pad, -1, np.int64)])
        parts.append((pi, ps))

    idx_p = np.concatenate([p[0] for p in parts])
    seg_p = np.concatenate([p[1] for p in parts])
    Kpad = len(idx_p)
    T = Kpad // 128
    tiles_per_bank = [len(p[0]) // 128 for p in parts]

    win_lo = np.zeros(T, np.int64)
    win_n = np.zeros(T, np.int64)
    t0 = 0
    for b in range(NBANKS):
        bank_lo = b * BANK
        bank_hi = min((b + 1) * BANK, P) - 1
        cover = bank_lo - 1  # highest pathway column covered so far
        nt = tiles_per_bank[b]
        for tt in range(nt):
            t = t0 + tt
            s = seg_p[t * 128 : (t + 1) * 128]
            s = s[s >= 0]
            if len(s):
                lo = min(int(s.min()), cover + 1)
                hi = max(int(s.max()), cover + 1)
            else:
                lo = min(cover + 1, bank_hi)
                hi = lo
            if tt == nt - 1:
                hi = bank_hi
            win_lo[t] = lo
            win_n[t] = hi - lo + 1
            cover = hi
        t0 += nt

    Wmax = int(win_n.max())
    assert Wmax <= BANK

    S = np.zeros((Kpad, Wmax), np.float16)
    valid = seg_p >= 0
    tk = np.arange(Kpad) // 128
    S[np.arange(Kpad)[valid], (seg_p - win_lo[tk])[valid]] = 1.0
    S_sb = np.ascontiguousarray(
        S.reshape(T, 128, Wmax).transpose(1, 0, 2).reshape(128, T * Wmax)
    )

    # dma_gather index layout: index j lives at partition j%16, slot j//16,
    # replicated across the 8 groups of 16 partitions.
    idx16 = idx_p.astype(np.int16)
    idx_sb = np.ascontiguousarray(np.tile(idx16.reshape(Kpad // 16, 16).T, (8, 1)))
    return idx_sb, S_sb, win_lo, win_n, Wmax, T, tiles_per_bank


def _build_program(Kpad, T, Wmax, tiles_per_bank, win_lo, win_n,
                   skip_gather=False, skip_mm=False):
    nc = bacc.Bacc("TRN2", target_bir_lowering=False, debug=False, num_devices=NCORES,
                   num_swdge_queues=NQ)
    f16, f32, i16 = mybir.dt.float16, mybir.dt.float32, mybir.dt.int16

    xt_d = nc.dram_tensor("xt", [G, BC], f16, kind="ExternalInput")
    idx_d = nc.dram_tensor("idx", [128, Kpad // 16], i16, kind="ExternalInput")
    s_d = nc.dram_tensor("smat", [128, T * Wmax], f16, kind="ExternalInput")
    inv_d = nc.dram_tensor("invsz", [128, P], f32, kind="ExternalInput")
    out_d = nc.dram_tensor("out", [BC, P], f32, kind="ExternalOutput")

    bank_of_tile = np.repeat(np.arange(NBANKS), tiles_per_bank)
    first_tile = {}
    last_tile = {}
    for t in range(T):
        b = int(bank_of_tile[t])
        first_tile.setdefault(b, t)
        last_tile[b] = t

    with tile.TileContext(nc) as tc:
        with (
            tc.tile_pool(name="const", bufs=1) as cpool,
            tc.tile_pool(name="gather", bufs=6) as gpool,
            tc.tile_pool(name="psum", bufs=1, space="PSUM") as ppool,
            tc.tile_pool(name="outp", bufs=1) as opool,
        ):
            idx_sb = cpool.tile([128, Kpad // 16], i16, tag="idx")
            nc.sync.dma_start(idx_sb[:], idx_d.ap())
            s_sb = cpool.tile([128, T * Wmax], f16, tag="smat")
            nc.sync.dma_start(s_sb[:], s_d.ap())
            inv_sb = cpool.tile([128, P], f32, tag="invsz")
            nc.sync.dma_start(inv_sb[:], inv_d.ap())

            psums = [
                [
                    ppool.tile(
                        [128, BANK], f32, tag=f"ps{h}{b}", name=f"psum{h}{b}"
                    )
                    for b in range(NBANKS)
                ]
                for h in range(2)
            ]

            # Zero every psum bank with a K=1 all-zero matmul (start=True
            # clears the bank's has_written bits and writes 0 everywhere) so
            # the window matmuls below can accumulate with start=False even
            # when their column windows partially overlap.
            zl = cpool.tile([1, 128], f16, tag="zl")
            nc.gpsimd.memset(zl[:], 0.0)
            zr = cpool.tile([1, BANK], f16, tag="zr")
            nc.gpsimd.memset(zr[:], 0.0)
            for h in range(2):
                for b in range(NBANKS):
                    nc.tensor.matmul(
                        psums[h][b][:, 0:BANK], zl[:], zr[:], start=True, stop=False
                    )

            nchunks = (T + CH - 1) // CH
            for c in range(nchunks):
                t_begin = c * CH
                t_end = min(T, t_begin + CH)
                n_t = t_end - t_begin
                gt = gpool.tile([128, CH, BC], f16, tag="gt")
                n_idx = n_t * 128
                if not skip_gather:
                    nc.gpsimd.dma_gather(
                        gt[:, 0:n_t, :],
                        xt_d.ap(),
                        idx_sb[:, t_begin * 8 : t_begin * 8 + n_t * 8],
                        num_idxs=n_idx,
                        num_idxs_reg=n_idx,
                        elem_size=BC,
                        queue_num=c % NQ,
                    )
                else:
                    nc.gpsimd.memset(gt[:, 0:1, :], 0.0)
                if skip_mm:
                    continue
                for tt in range(t_begin, t_end):
                    b = int(bank_of_tile[tt])
                    off = int(win_lo[tt]) - b * BANK
                    n = int(win_n[tt])
                    for h in range(2):
                        nc.tensor.matmul(
                            psums[h][b][:, off : off + n],
                            gt[:, tt - t_begin, h * 128 : (h + 1) * 128],
                            s_sb[:, tt * Wmax : tt * Wmax + n],
                            start=False,
                            stop=(tt == last_tile[b]),
                        )

            for h in range(2):
                for b in range(NBANKS):
                    bw = min(BANK, P - b * BANK)
                    ot = opool.tile([128, bw], f32, tag=f"ot{h}{b}", name=f"ot{h}{b}")
                    nc.vector.tensor_mul(
                        ot[:], psums[h][b][:, 0:bw], inv_sb[:, b * BANK : b * BANK + bw]
                    )
                    nc.sync.dma_start(
                        out_d.ap()[h * 128 : (h + 1) * 128, b * BANK : b * BANK + bw],
                        ot[:],
                    )
    return nc


def _prepare(gene_set_features, flat_indices, segment_ids, segment_sizes):
    idx_sb, S_sb, win_lo, win_n, Wmax, T, tiles_per_bank = _build_schedule(
        flat_indices, segment_ids
    )
    Kpad = T * 128
    nc = _build_program(Kpad, T, Wmax, tiles_per_bank, win_lo, win_n)
    nc.compile()

    x = np.asarray(gene_set_features, dtype=np.float32)
    xt16 = np.ascontiguousarray(x.T.astype(np.float16))  # (G, B)
    inv = (1.0 / np.asarray(segment_sizes, dtype=np.float32)).astype(np.float32)
    inv_sb = np.ascontiguousarray(np.broadcast_to(inv[None, :], (128, P)))

    in_maps = []
    for c in range(NCORES):
        in_maps.append(
            {
                "xt": np.ascontiguousarray(xt16[:, c * BC : (c + 1) * BC]),
                "idx": idx_sb,
                "smat": S_sb,
                "invsz": inv_sb,
            }
        )
    return nc, in_maps


def kernel(gene_set_features, flat_indices, segment_ids, segment_sizes, _res_hook=None):
    nc, in_maps = _prepare(
        gene_set_features, flat_indices, segment_ids, segment_sizes
    )
    res = run_bass_kernel_spmd(nc, in_maps, list(range(NCORES)))
    if _res_hook is not None:
        _res_hook(res)
    out = np.concatenate(
        [np.asarray(res.results[c]["out"]) for c in range(NCORES)], axis=0
    )
    return out.astype(np.float32)


# revision 12
# speedup vs baseline: 1.1094x; 1.1094x over previous
"""Trainium2 Bass kernel for CellPathwayPoolingAggregator (segment mean).

out[b, p] = (1/segment_sizes[p]) * sum_{k: segment_ids[k]==p} x[b, flat_indices[k]]

Strategy (8 cores, sharded by contiguous pathway ranges):
  - Host: transpose x -> xT (G, B) fp16 (one shared copy per core). Split the
    1000 pathways into 8 contiguous ranges (<=128 pathways each) with roughly
    equal index counts.
  - Device (per core): gpsimd.dma_gather pulls the core's ~K/8 indexed
    gene-rows (full 2048-batch rows, 4KB each -> full DMA bandwidth) from
    DRAM into SBUF laid out [k%128, k//128, b]. For each 128-row K-tile a PE
    matmul with the per-core one-hot matrix S (128 k-rows x 128 local
    pathways, stationary) streams the gathered rows (N=512 x 4) and
    accumulates pathway x batch sums into one 4-bank PSUM tile.
  - DVE scales rows by 1/segment_sizes (per-partition scalar), DMA stores
    the (128, 2048) f32 transposed output slice; host reassembles/transposes.

All data-dependence lives in the per-core idx/S tensors, so the single SPMD
program is uniform across cores (T_max tiles each, zero-padded).
"""

import sys

import numpy as np

_TRN_REPO = "/opt/trn_rl_repo"
if _TRN_REPO not in sys.path:
    sys.path.insert(0, _TRN_REPO)

import concourse.bass as bass  # noqa: F401  (AP helpers via bass)
import concourse.mybir as mybir
import concourse.tile as tile
from concourse import bacc
from concourse.bass_utils import run_bass_kernel_spmd

B, G, P = 2048, 10000, 1000
NCORES = 8
PC = 128          # max pathways per core (psum partition dim)
NB = B // 512     # matmul N-slices per K-tile (4 banks of 512 f32)
CH = 8            # gather-chunk size in 128-index tiles (<=1024 idxs per dma_gather)
NQ = 4            # SWDGE queues, round-robin per chunk


def _split_ranges(seg_sorted):
    """Contiguous pathway ranges, <=128 pathways each, ~equal index counts."""
    K = len(seg_sorted)
    cnt = np.bincount(seg_sorted, minlength=P)
    cum = np.concatenate([[0], np.cumsum(cnt)])  # cum[p] = #entries below p
    bounds = [0]
    for c in range(1, NCORES):
        target = K * c // NCORES
        b = int(np.searchsorted(cum, target))
        b = max(bounds[-1] + 1, min(b, bounds[-1] + PC))
        b = max(b, P - PC * (NCORES - c))  # leave room for remaining cores
        b = min(b, P - (NCORES - c))       # leave >=1 pathway per core
        bounds.append(b)
    bounds.append(P)
    return bounds


def _build_schedule(flat_indices, segment_ids):
    seg = np.asarray(segment_ids, dtype=np.int64)
    idx = np.asarray(flat_indices, dtype=np.int64)
    order = np.argsort(seg, kind="stable")
    seg = seg[order]
    idx = idx[order]

    bounds = _split_ranges(seg)
    cores = []
    for c in range(NCORES):
        lo_p, hi_p = bounds[c], bounds[c + 1]
        lo = np.searchsorted(seg, lo_p, side="left")
        hi = np.searchsorted(seg, hi_p, side="left")
        cores.append((lo_p, hi_p, idx[lo:hi], seg[lo:hi] - lo_p))

    T = max(1, max((len(ci) + 127) // 128 for _, _, ci, _ in cores))
    Kpad = T * 128

    idx_sbs, s_sbs = [], []
    for lo_p, hi_p, ci, cols in cores:
        pad = Kpad - len(ci)
        idx_p = np.concatenate([ci, np.zeros(pad, np.int64)])
        col_p = np.concatenate([cols, np.full(pad, -1, np.int64)])
        S = np.zeros((Kpad, PC), np.float16)
        valid = col_p >= 0
        S[np.arange(Kpad)[valid], col_p[valid]] = 1.0
        s_sbs.append(
            np.ascontiguousarray(
                S.reshape(T, 128, PC).transpose(1, 0, 2).reshape(128, T * PC)
            )
        )
        idx16 = idx_p.astype(np.int16)
        idx_sbs.append(
            np.ascontiguousarray(np.tile(idx16.reshape(Kpad // 16, 16).T, (8, 1)))
        )
    return bounds, idx_sbs, s_sbs, T


def _build_program(T):
    nc = bacc.Bacc(
        "TRN2",
        target_bir_lowering=False,
        debug=False,
        num_devices=NCORES,
        num_swdge_queues=NQ,
    )
    f16, f32, i16 = mybir.dt.float16, mybir.dt.float32, mybir.dt.int16

    xt_d = nc.dram_tensor("xt", [G, B], f16, kind="ExternalInput")
    idx_d = nc.dram_tensor("idx", [128, T * 8], i16, kind="ExternalInput")
    s_d = nc.dram_tensor("smat", [128, T * PC], f16, kind="ExternalInput")
    inv_d = nc.dram_tensor("invsz", [128, 1], f32, kind="ExternalInput")
    out_d = nc.dram_tensor("out", [PC, B], f32, kind="ExternalOutput")

    with tile.TileContext(nc) as tc:
        with (
            tc.tile_pool(name="const", bufs=1) as cpool,
            tc.tile_pool(name="gather", bufs=3) as gpool,
            tc.tile_pool(name="psum", bufs=1, space="PSUM") as ppool,
            tc.tile_pool(name="outp", bufs=1) as opool,
        ):
            idx_sb = cpool.tile([128, T * 8], i16, tag="idx")
            nc.sync.dma_start(idx_sb[:], idx_d.ap())
            s_sb = cpool.tile([128, T * PC], f16, tag="smat")
            nc.sync.dma_start(s_sb[:], s_d.ap())
            inv_sb = cpool.tile([128, 1], f32, tag="invsz")
            nc.sync.dma_start(inv_sb[:], inv_d.ap())

            ps = ppool.tile([128, B], f32, tag="ps", name="ps")

            # Zero all four psum banks (start=True clears has_written per
            # bank) so the accumulating matmuls below can use start=False.
            zl = cpool.tile([1, 128], f16, tag="zl")
            nc.gpsimd.memset(zl[:], 0.0)
            zr = cpool.tile([1, 512], f16, tag="zr")
            nc.gpsimd.memset(zr[:], 0.0)
            for n in range(NB):
                nc.tensor.matmul(
                    ps[:, n * 512 : (n + 1) * 512], zl[:], zr[:],
                    start=True, stop=False,
                )

            nchunks = (T + CH - 1) // CH
            for c in range(nchunks):
                t_begin = c * CH
                t_end = min(T, t_begin + CH)
                n_t = t_end - t_begin
                gt = gpool.tile([128, CH, B], f16, tag="gt")
                n_idx = n_t * 128
                nc.gpsimd.dma_gather(
                    gt[:, 0:n_t, :],
                    xt_d.ap(),
                    idx_sb[:, t_begin * 8 : t_begin * 8 + n_t * 8],
                    num_idxs=n_idx,
                    num_idxs_reg=n_idx,
                    elem_size=B,
                    queue_num=c % NQ,
                )
                for tt in range(t_begin, t_end):
                    lhsT = s_sb[:, tt * PC : (tt + 1) * PC]
                    for n in range(NB):
                        nc.tensor.matmul(
                            ps[:, n * 512 : (n + 1) * 512],
                            lhsT,
                            gt[:, tt - t_begin, n * 512 : (n + 1) * 512],
                            start=False,
                            stop=(tt == T - 1),
                        )

            ot = opool.tile([128, B], f32, tag="ot", name="ot")
            nc.vector.tensor_scalar_mul(ot[:], ps[:], inv_sb[:])
            nc.sync.dma_start(out_d.ap(), ot[:])
    return nc


def _prepare(gene_set_features, flat_indices, segment_ids, segment_sizes):
    bounds, idx_sbs, s_sbs, T = _build_schedule(flat_indices, segment_ids)
    nc = _build_program(T)
    nc.compile()

    x = np.asarray(gene_set_features, dtype=np.float32)
    xt16 = np.ascontiguousarray(x.T.astype(np.float16))  # (G, B)
    sizes = np.asarray(segment_sizes, dtype=np.float32)

    in_maps = []
    for c in range(NCORES):
        lo_p, hi_p = bounds[c], bounds[c + 1]
        inv = np.ones((128, 1), np.float32)
        inv[: hi_p - lo_p, 0] = 1.0 / sizes[lo_p:hi_p]
        in_maps.append(
            {"xt": xt16, "idx": idx_sbs[c], "smat": s_sbs[c], "invsz": inv}
        )
    return nc, in_maps, bounds


def kernel(gene_set_features, flat_indices, segment_ids, segment_sizes, _res_hook=None):
    nc, in_maps, bounds = _prepare(
        gene_set_features, flat_indices, segment_ids, segment_sizes
    )
    res = run_bass_kernel_spmd(nc, in_maps, list(range(NCORES)))
    if _res_hook is not None:
        _res_hook(res)
    outT = np.empty((P, B), np.float32)
    for c in range(NCORES):
        lo_p, hi_p = bounds[c], bounds[c + 1]
        outT[lo_p:hi_p] = np.asarray(res.results[c]["out"])[: hi_p - lo_p]
    return np.ascontiguousarray(outT.T)


# revision 14
# speedup vs baseline: 1.3400x; 1.2078x over previous
"""Trainium2 Bass kernel for CellPathwayPoolingAggregator (segment mean).

out[b, p] = (1/segment_sizes[p]) * sum_{k: segment_ids[k]==p} x[b, flat_indices[k]]

Strategy (8 cores, sharded by contiguous pathway ranges):
  - Host: transpose x -> xT (G, B) fp16 (one shared copy per core). Split the
    1000 pathways into 8 contiguous ranges (<=128 pathways each) with roughly
    equal index counts.
  - Device (per core): gpsimd.dma_gather pulls the core's ~K/8 indexed
    gene-rows (full 2048-batch rows, 4KB each -> full DMA bandwidth) from
    DRAM into SBUF laid out [k%128, k//128, b]. For each 128-row K-tile a PE
    matmul with the per-core one-hot matrix S (128 k-rows x 128 local
    pathways, stationary) streams the gathered rows (N=512 x 4) and
    accumulates pathway x batch sums into one 4-bank PSUM tile.
  - DVE scales rows by 1/segment_sizes (per-partition scalar), DMA stores
    the (128, 2048) f32 transposed output slice; host reassembles/transposes.

All data-dependence lives in the per-core idx/S tensors, so the single SPMD
program is uniform across cores (T_max tiles each, zero-padded).
"""

import sys

import numpy as np

_TRN_REPO = "/opt/trn_rl_repo"
if _TRN_REPO not in sys.path:
    sys.path.insert(0, _TRN_REPO)

import concourse.bass as bass  # noqa: F401  (AP helpers via bass)
import concourse.mybir as mybir
import concourse.tile as tile
from concourse import bacc
from concourse.bass_utils import run_bass_kernel_spmd

B, G, P = 2048, 10000, 1000
NCORES = 8
PC = 128          # max pathways per core (psum partition dim)
NB = B // 512     # matmul N-slices per K-tile (4 banks of 512 f32)
CH = 8            # gather-chunk size in 128-index tiles (<=1024 idxs per dma_gather)
NQ = 4            # SWDGE queues, round-robin per chunk


def _split_ranges(seg_sorted):
    """Contiguous pathway ranges, <=128 pathways each, ~equal index counts."""
    K = len(seg_sorted)
    cnt = np.bincount(seg_sorted, minlength=P)
    cum = np.concatenate([[0], np.cumsum(cnt)])  # cum[p] = #entries below p
    bounds = [0]
    for c in range(1, NCORES):
        target = K * c // NCORES
        b = int(np.searchsorted(cum, target))
        b = max(bounds[-1] + 1, min(b, bounds[-1] + PC))
        b = max(b, P - PC * (NCORES - c))  # leave room for remaining cores
        b = min(b, P - (NCORES - c))       # leave >=1 pathway per core
        bounds.append(b)
    bounds.append(P)
    return bounds


def _build_schedule(flat_indices, segment_ids):
    seg = np.asarray(segment_ids, dtype=np.int64)
    idx = np.asarray(flat_indices, dtype=np.int64)
    order = np.argsort(seg, kind="stable")
    seg = seg[order]
    idx = idx[order]

    bounds = _split_ranges(seg)
    cores = []
    for c in range(NCORES):
        lo_p, hi_p = bounds[c], bounds[c + 1]
        lo = np.searchsorted(seg, lo_p, side="left")
        hi = np.searchsorted(seg, hi_p, side="left")
        # Deduplicate gene rows within the core: each distinct gene is
        # gathered once; S accumulates per-(gene,pathway) counts (exact in
        # fp16 for the counts seen here).
        uidx, inv = np.unique(idx[lo:hi], return_inverse=True)
        cores.append((lo_p, hi_p, uidx, inv, seg[lo:hi] - lo_p))

    T = max(1, max((len(u) + 127) // 128 for _, _, u, _, _ in cores))
    Kpad = T * 128

    idx_sbs, s_sbs = [], []
    for lo_p, hi_p, uidx, inv, cols in cores:
        nu = len(uidx)
        idx_p = np.concatenate([uidx, np.zeros(Kpad - nu, np.int64)])
        S = np.zeros((Kpad, PC), np.float32)
        np.add.at(S, (inv, cols), 1.0)
        S = S.astype(np.float16)
        s_sbs.append(
            np.ascontiguousarray(
                S.reshape(T, 128, PC).transpose(1, 0, 2).reshape(128, T * PC)
            )
        )
        idx16 = idx_p.astype(np.int16)
        idx_sbs.append(
            np.ascontiguousarray(np.tile(idx16.reshape(Kpad // 16, 16).T, (8, 1)))
        )
    return bounds, idx_sbs, s_sbs, T


def _build_program(T):
    nc = bacc.Bacc(
        "TRN2",
        target_bir_lowering=False,
        debug=False,
        num_devices=NCORES,
        num_swdge_queues=NQ,
    )
    f16, f32, i16 = mybir.dt.float16, mybir.dt.float32, mybir.dt.int16

    xt_d = nc.dram_tensor("xt", [G, B], f16, kind="ExternalInput")
    idx_d = nc.dram_tensor("idx", [128, T * 8], i16, kind="ExternalInput")
    s_d = nc.dram_tensor("smat", [128, T * PC], f16, kind="ExternalInput")
    inv_d = nc.dram_tensor("invsz", [128, 1], f32, kind="ExternalInput")
    out_d = nc.dram_tensor("out", [PC, B], f32, kind="ExternalOutput")

    with tile.TileContext(nc) as tc:
        with (
            tc.tile_pool(name="const", bufs=1) as cpool,
            tc.tile_pool(name="gather", bufs=3) as gpool,
            tc.tile_pool(name="psum", bufs=1, space="PSUM") as ppool,
            tc.tile_pool(name="outp", bufs=1) as opool,
        ):
            idx_sb = cpool.tile([128, T * 8], i16, tag="idx")
            nc.sync.dma_start(idx_sb[:], idx_d.ap())
            s_sb = cpool.tile([128, T * PC], f16, tag="smat")
            nc.sync.dma_start(s_sb[:], s_d.ap())
            inv_sb = cpool.tile([128, 1], f32, tag="invsz")
            nc.sync.dma_start(inv_sb[:], inv_d.ap())

            ps = ppool.tile([128, B], f32, tag="ps", name="ps")

            # Zero all four psum banks (start=True clears has_written per
            # bank) so the accumulating matmuls below can use start=False.
            # memsets on DVE (not Pool) so Pool's first instruction is the
            # GpSimd ucode library reload, overlapping it with const loads.
            zl = cpool.tile([1, 128], f16, tag="zl")
            nc.vector.memset(zl[:], 0.0)
            zr = cpool.tile([1, 512], f16, tag="zr")
            nc.vector.memset(zr[:], 0.0)
            for n in range(NB):
                nc.tensor.matmul(
                    ps[:, n * 512 : (n + 1) * 512], zl[:], zr[:],
                    start=True, stop=False,
                )

            nchunks = (T + CH - 1) // CH
            for c in range(nchunks):
                t_begin = c * CH
                t_end = min(T, t_begin + CH)
                n_t = t_end - t_begin
                gt = gpool.tile([128, CH, B], f16, tag="gt")
                n_idx = n_t * 128
                nc.gpsimd.dma_gather(
                    gt[:, 0:n_t, :],
                    xt_d.ap(),
                    idx_sb[:, t_begin * 8 : t_begin * 8 + n_t * 8],
                    num_idxs=n_idx,
                    num_idxs_reg=n_idx,
                    elem_size=B,
                    queue_num=c % NQ,
                )
                for tt in range(t_begin, t_end):
                    lhsT = s_sb[:, tt * PC : (tt + 1) * PC]
                    for n in range(NB):
                        nc.tensor.matmul(
                            ps[:, n * 512 : (n + 1) * 512],
                            lhsT,
                            gt[:, tt - t_begin, n * 512 : (n + 1) * 512],
                            start=False,
                            stop=(tt == T - 1),
                        )

            ot = opool.tile([128, B], f32, tag="ot", name="ot")
            nc.vector.tensor_scalar_mul(ot[:], ps[:], inv_sb[:])
            nc.sync.dma_start(out_d.ap(), ot[:])
    return nc


def _prepare(gene_set_features, flat_indices, segment_ids, segment_sizes):
    bounds, idx_sbs, s_sbs, T = _build_schedule(flat_indices, segment_ids)
    nc = _build_program(T)
    nc.compile()

    x = np.asarray(gene_set_features, dtype=np.float32)
    xt16 = np.ascontiguousarray(x.T.astype(np.float16))  # (G, B)
    sizes = np.asarray(segment_sizes, dtype=np.float32)

    in_maps = []
    for c in range(NCORES):
        lo_p, hi_p = bounds[c], bounds[c + 1]
        inv = np.ones((128, 1), np.float32)
        inv[: hi_p - lo_p, 0] = 1.0 / sizes[lo_p:hi_p]
        in_maps.append(
            {"xt": xt16, "idx": idx_sbs[c], "smat": s_sbs[c], "invsz": inv}
        )
    return nc, in_maps, bounds


def kernel(gene_set_features, flat_indices, segment_ids, segment_sizes, _res_hook=None):
    nc, in_maps, bounds = _prepare(
        gene_set_features, flat_indices, segment_ids, segment_sizes
    )
    res = run_bass_kernel_spmd(nc, in_maps, list(range(NCORES)))
    if _res_hook is not None:
        _res_hook(res)
    outT = np.empty((P, B), np.float32)
    for c in range(NCORES):
        lo_p, hi_p = bounds[c], bounds[c + 1]
        outT[lo_p:hi_p] = np.asarray(res.results[c]["out"])[: hi_p - lo_p]
    return np.ascontiguousarray(outT.T)


# revision 15
# speedup vs baseline: 1.5292x; 1.1412x over previous
"""Trainium2 Bass kernel for CellPathwayPoolingAggregator (segment mean).

out[b, p] = (1/segment_sizes[p]) * sum_{k: segment_ids[k]==p} x[b, flat_indices[k]]

Strategy (8 cores, sharded by contiguous pathway ranges):
  - Host: transpose x -> xT (G, B) fp16 (one shared copy per core). Split the
    1000 pathways into 8 contiguous ranges (<=128 pathways each) with roughly
    equal index counts.
  - Device (per core): gpsimd.dma_gather pulls the core's ~K/8 indexed
    gene-rows (full 2048-batch rows, 4KB each -> full DMA bandwidth) from
    DRAM into SBUF laid out [k%128, k//128, b]. For each 128-row K-tile a PE
    matmul with the per-core one-hot matrix S (128 k-rows x 128 local
    pathways, stationary) streams the gathered rows (N=512 x 4) and
    accumulates pathway x batch sums into one 4-bank PSUM tile.
  - DVE scales rows by 1/segment_sizes (per-partition scalar), DMA stores
    the (128, 2048) f32 transposed output slice; host reassembles/transposes.

All data-dependence lives in the per-core idx/S tensors, so the single SPMD
program is uniform across cores (T_max tiles each, zero-padded).
"""

import sys

import numpy as np

_TRN_REPO = "/opt/trn_rl_repo"
if _TRN_REPO not in sys.path:
    sys.path.insert(0, _TRN_REPO)

import concourse.bass as bass  # noqa: F401  (AP helpers via bass)
import concourse.mybir as mybir
import concourse.tile as tile
from concourse import bacc
from concourse.bass_utils import run_bass_kernel_spmd

B, G, P = 2048, 10000, 1000
NCORES = 8
PC = 128          # max pathways per core (psum partition dim)
NB = B // 512     # matmul N-slices per K-tile (4 banks of 512 f32)
CH = 8            # gather-chunk size in 128-index tiles (<=1024 idxs per dma_gather)
NQ = 4            # SWDGE queues, round-robin per chunk


def _split_ranges(seg_sorted):
    """Contiguous pathway ranges, <=128 pathways each, ~equal index counts."""
    K = len(seg_sorted)
    cnt = np.bincount(seg_sorted, minlength=P)
    cum = np.concatenate([[0], np.cumsum(cnt)])  # cum[p] = #entries below p
    bounds = [0]
    for c in range(1, NCORES):
        target = K * c // NCORES
        b = int(np.searchsorted(cum, target))
        b = max(bounds[-1] + 1, min(b, bounds[-1] + PC))
        b = max(b, P - PC * (NCORES - c))  # leave room for remaining cores
        b = min(b, P - (NCORES - c))       # leave >=1 pathway per core
        bounds.append(b)
    bounds.append(P)
    return bounds


def _build_schedule(flat_indices, segment_ids):
    seg = np.asarray(segment_ids, dtype=np.int64)
    idx = np.asarray(flat_indices, dtype=np.int64)
    order = np.argsort(seg, kind="stable")
    seg = seg[order]
    idx = idx[order]

    bounds = _split_ranges(seg)
    cores = []
    for c in range(NCORES):
        lo_p, hi_p = bounds[c], bounds[c + 1]
        lo = np.searchsorted(seg, lo_p, side="left")
        hi = np.searchsorted(seg, hi_p, side="left")
        # Deduplicate gene rows within the core: each distinct gene is
        # gathered once; S accumulates per-(gene,pathway) counts (exact in
        # fp16 for the counts seen here).
        uidx, inv = np.unique(idx[lo:hi], return_inverse=True)
        cores.append((lo_p, hi_p, uidx, inv, seg[lo:hi] - lo_p))

    T = max(1, max((len(u) + 127) // 128 for _, _, u, _, _ in cores))
    Kpad = T * 128

    idx_sbs, s_sbs = [], []
    for lo_p, hi_p, uidx, inv, cols in cores:
        nu = len(uidx)
        idx_p = np.concatenate([uidx, np.zeros(Kpad - nu, np.int64)])
        S = np.zeros((Kpad, PC), np.float32)
        np.add.at(S, (inv, cols), 1.0)
        S = S.astype(np.float16)
        s_sbs.append(
            np.ascontiguousarray(
                S.reshape(T, 128, PC).transpose(1, 0, 2).reshape(128, T * PC)
            )
        )
        idx16 = idx_p.astype(np.int16)
        idx_sbs.append(
            np.ascontiguousarray(np.tile(idx16.reshape(Kpad // 16, 16).T, (8, 1)))
        )
    return bounds, idx_sbs, s_sbs, T


def _build_program(T):
    nc = bacc.Bacc(
        "TRN2",
        target_bir_lowering=False,
        debug=False,
        num_devices=NCORES,
        num_swdge_queues=NQ,
    )
    f16, f32, i16 = mybir.dt.float16, mybir.dt.float32, mybir.dt.int16

    xt_d = nc.dram_tensor("xt", [G, B], f16, kind="ExternalInput")
    idx_d = nc.dram_tensor("idx", [128, T * 8], i16, kind="ExternalInput")
    s_d = nc.dram_tensor("smat", [128, T * PC], f16, kind="ExternalInput")
    inv_d = nc.dram_tensor("invsz", [128, 1], f32, kind="ExternalInput")
    out_d = nc.dram_tensor("out", [PC, B], f32, kind="ExternalOutput")

    with tile.TileContext(nc) as tc:
        with (
            tc.tile_pool(name="const", bufs=1) as cpool,
            tc.tile_pool(name="gather", bufs=3) as gpool,
            tc.tile_pool(name="psum", bufs=1, space="PSUM") as ppool,
            tc.tile_pool(name="outp", bufs=1) as opool,
        ):
            idx_sb = cpool.tile([128, T * 8], i16, tag="idx")
            nc.sync.dma_start(idx_sb[:], idx_d.ap())
            s_sb = cpool.tile([128, T * PC], f16, tag="smat")
            nc.sync.dma_start(s_sb[:], s_d.ap())
            inv_sb = cpool.tile([128, 1], f32, tag="invsz")
            nc.sync.dma_start(inv_sb[:], inv_d.ap())

            ps = ppool.tile([128, B], f32, tag="ps", name="ps")

            # Zero all four psum banks (start=True clears has_written per
            # bank) so the accumulating matmuls below can use start=False.
            # memsets on DVE (not Pool) so Pool's first instruction is the
            # GpSimd ucode library reload, overlapping it with const loads.
            zl = cpool.tile([1, 128], f16, tag="zl")
            nc.vector.memset(zl[:], 0.0)
            zr = cpool.tile([1, 512], f16, tag="zr")
            nc.vector.memset(zr[:], 0.0)
            for n in range(NB):
                nc.tensor.matmul(
                    ps[:, n * 512 : (n + 1) * 512], zl[:], zr[:],
                    start=True, stop=False,
                )

            # Ramp-up chunk sizes: small first chunks so PE gets data early
            # (the first gather sits behind the GpSimd library load), then
            # full CH-tile chunks (1024-idx dma_gather cap).
            chunks = []
            t_begin = 0
            for size in (1, 2, 4):
                if t_begin < T:
                    n_t = min(size, T - t_begin)
                    chunks.append((t_begin, n_t))
                    t_begin += n_t
            while t_begin < T:
                n_t = min(CH, T - t_begin)
                chunks.append((t_begin, n_t))
                t_begin += n_t

            for c, (t_begin, n_t) in enumerate(chunks):
                gt = gpool.tile([128, CH, B], f16, tag="gt")
                n_idx = n_t * 128
                nc.gpsimd.dma_gather(
                    gt[:, 0:n_t, :],
                    xt_d.ap(),
                    idx_sb[:, t_begin * 8 : t_begin * 8 + n_t * 8],
                    num_idxs=n_idx,
                    num_idxs_reg=n_idx,
                    elem_size=B,
                    queue_num=c % NQ,
                )
                for tt in range(t_begin, t_begin + n_t):
                    lhsT = s_sb[:, tt * PC : (tt + 1) * PC]
                    for n in range(NB):
                        nc.tensor.matmul(
                            ps[:, n * 512 : (n + 1) * 512],
                            lhsT,
                            gt[:, tt - t_begin, n * 512 : (n + 1) * 512],
                            start=False,
                            stop=(tt == T - 1),
                        )

            # Per-bank eviction + store so bank n ships while bank n+1's last
            # matmul is still draining.
            for n in range(NB):
                ot = opool.tile([128, 512], f32, tag=f"ot{n}", name=f"ot{n}")
                nc.vector.tensor_scalar_mul(
                    ot[:], ps[:, n * 512 : (n + 1) * 512], inv_sb[:]
                )
                nc.sync.dma_start(out_d.ap()[:, n * 512 : (n + 1) * 512], ot[:])
    return nc


def _prepare(gene_set_features, flat_indices, segment_ids, segment_sizes):
    bounds, idx_sbs, s_sbs, T = _build_schedule(flat_indices, segment_ids)
    nc = _build_program(T)
    nc.compile()

    x = np.asarray(gene_set_features, dtype=np.float32)
    xt16 = np.ascontiguousarray(x.T.astype(np.float16))  # (G, B)
    sizes = np.asarray(segment_sizes, dtype=np.float32)

    in_maps = []
    for c in range(NCORES):
        lo_p, hi_p = bounds[c], bounds[c + 1]
        inv = np.ones((128, 1), np.float32)
        inv[: hi_p - lo_p, 0] = 1.0 / sizes[lo_p:hi_p]
        in_maps.append(
            {"xt": xt16, "idx": idx_sbs[c], "smat": s_sbs[c], "invsz": inv}
        )
    return nc, in_maps, bounds


def kernel(gene_set_features, flat_indices, segment_ids, segment_sizes, _res_hook=None):
    nc, in_maps, bounds = _prepare(
        gene_set_features, flat_indices, segment_ids, segment_sizes
    )
    res = run_bass_kernel_spmd(nc, in_maps, list(range(NCORES)))
    if _res_hook is not None:
        _res_hook(res)
    outT = np.empty((P, B), np.float32)
    for c in range(NCORES):
        lo_p, hi_p = bounds[c], bounds[c + 1]
        outT[lo_p:hi_p] = np.asarray(res.results[c]["out"])[: hi_p - lo_p]
    return np.ascontiguousarray(outT.T)


# revision 18
# speedup vs baseline: 1.5463x; 1.0112x over previous
"""Trainium2 Bass kernel for CellPathwayPoolingAggregator (segment mean).

out[b, p] = (1/segment_sizes[p]) * sum_{k: segment_ids[k]==p} x[b, flat_indices[k]]

Strategy (8 cores, sharded by contiguous pathway ranges):
  - Host: transpose x -> xT (G, B) fp16 (one shared copy per core). Split the
    1000 pathways into 8 contiguous ranges (<=128 pathways each) with roughly
    equal index counts.
  - Device (per core): gpsimd.dma_gather pulls the core's ~K/8 indexed
    gene-rows (full 2048-batch rows, 4KB each -> full DMA bandwidth) from
    DRAM into SBUF laid out [k%128, k//128, b]. For each 128-row K-tile a PE
    matmul with the per-core one-hot matrix S (128 k-rows x 128 local
    pathways, stationary) streams the gathered rows (N=512 x 4) and
    accumulates pathway x batch sums into one 4-bank PSUM tile.
  - DVE scales rows by 1/segment_sizes (per-partition scalar), DMA stores
    the (128, 2048) f32 transposed output slice; host reassembles/transposes.

All data-dependence lives in the per-core idx/S tensors, so the single SPMD
program is uniform across cores (T_max tiles each, zero-padded).
"""

import sys

import numpy as np

_TRN_REPO = "/opt/trn_rl_repo"
if _TRN_REPO not in sys.path:
    sys.path.insert(0, _TRN_REPO)

import concourse.bass as bass  # noqa: F401  (AP helpers via bass)
import concourse.mybir as mybir
import concourse.tile as tile
from concourse import bacc
from concourse.bass_utils import run_bass_kernel_spmd

B, G, P = 2048, 10000, 1000
NCORES = 8
PC = 128          # max pathways per core (psum partition dim)
NB = B // 512     # matmul N-slices per K-tile (4 banks of 512 f32)
CH = 4            # gather-chunk size in 128-index tiles (<=1024 idxs per dma_gather)
NQ = 4            # SWDGE queues, round-robin per chunk


def _split_ranges(seg_sorted):
    """Contiguous pathway ranges, <=128 pathways each, ~equal index counts."""
    K = len(seg_sorted)
    cnt = np.bincount(seg_sorted, minlength=P)
    cum = np.concatenate([[0], np.cumsum(cnt)])  # cum[p] = #entries below p
    bounds = [0]
    for c in range(1, NCORES):
        target = K * c // NCORES
        b = int(np.searchsorted(cum, target))
        b = max(bounds[-1] + 1, min(b, bounds[-1] + PC))
        b = max(b, P - PC * (NCORES - c))  # leave room for remaining cores
        b = min(b, P - (NCORES - c))       # leave >=1 pathway per core
        bounds.append(b)
    bounds.append(P)
    return bounds


def _build_schedule(flat_indices, segment_ids):
    seg = np.asarray(segment_ids, dtype=np.int64)
    idx = np.asarray(flat_indices, dtype=np.int64)
    order = np.argsort(seg, kind="stable")
    seg = seg[order]
    idx = idx[order]

    bounds = _split_ranges(seg)
    cores = []
    for c in range(NCORES):
        lo_p, hi_p = bounds[c], bounds[c + 1]
        lo = np.searchsorted(seg, lo_p, side="left")
        hi = np.searchsorted(seg, hi_p, side="left")
        # Deduplicate gene rows within the core: each distinct gene is
        # gathered once; S accumulates per-(gene,pathway) counts (exact in
        # fp16 for the counts seen here).
        uidx, inv = np.unique(idx[lo:hi], return_inverse=True)
        cores.append((lo_p, hi_p, uidx, inv, seg[lo:hi] - lo_p))

    T = max(1, max((len(u) + 127) // 128 for _, _, u, _, _ in cores))
    Kpad = T * 128

    idx_sbs, s_sbs = [], []
    for lo_p, hi_p, uidx, inv, cols in cores:
        nu = len(uidx)
        idx_p = np.concatenate([uidx, np.zeros(Kpad - nu, np.int64)])
        S = np.zeros((Kpad, PC), np.float32)
        np.add.at(S, (inv, cols), 1.0)
        S = S.astype(np.float16)
        s_sbs.append(
            np.ascontiguousarray(
                S.reshape(T, 128, PC).transpose(1, 0, 2).reshape(128, T * PC)
            )
        )
        idx16 = idx_p.astype(np.int16)
        idx_sbs.append(
            np.ascontiguousarray(np.tile(idx16.reshape(Kpad // 16, 16).T, (8, 1)))
        )
    return bounds, idx_sbs, s_sbs, T


def _build_program(T):
    nc = bacc.Bacc(
        "TRN2",
        target_bir_lowering=False,
        debug=False,
        num_devices=NCORES,
        num_swdge_queues=NQ,
    )
    f16, f32, i16 = mybir.dt.float16, mybir.dt.float32, mybir.dt.int16

    xt_d = nc.dram_tensor("xt", [G, B], f16, kind="ExternalInput")
    idx_d = nc.dram_tensor("idx", [128, T * 8], i16, kind="ExternalInput")
    s_d = nc.dram_tensor("smat", [128, T * PC], f16, kind="ExternalInput")
    inv_d = nc.dram_tensor("invsz", [128, 1], f32, kind="ExternalInput")
    out_d = nc.dram_tensor("out", [PC, B], f32, kind="ExternalOutput")

    with tile.TileContext(nc) as tc:
        with (
            tc.tile_pool(name="const", bufs=1) as cpool,
            tc.tile_pool(name="gather", bufs=6) as gpool,
            tc.tile_pool(name="psum", bufs=1, space="PSUM") as ppool,
            tc.tile_pool(name="outp", bufs=1) as opool,
        ):
            idx_sb = cpool.tile([128, T * 8], i16, tag="idx")
            nc.sync.dma_start(idx_sb[:], idx_d.ap())
            s_sb = cpool.tile([128, T * PC], f16, tag="smat")
            nc.sync.dma_start(s_sb[:], s_d.ap())
            inv_sb = cpool.tile([128, 1], f32, tag="invsz")
            nc.sync.dma_start(inv_sb[:], inv_d.ap())

            ps = ppool.tile([128, B], f32, tag="ps", name="ps")

            # Zero all four psum banks (start=True clears has_written per
            # bank) so the accumulating matmuls below can use start=False.
            # memsets on DVE (not Pool) so Pool's first instruction is the
            # GpSimd ucode library reload, overlapping it with const loads.
            zl = cpool.tile([1, 128], f16, tag="zl")
            nc.vector.memset(zl[:], 0.0)
            zr = cpool.tile([1, 512], f16, tag="zr")
            nc.vector.memset(zr[:], 0.0)
            for n in range(NB):
                nc.tensor.matmul(
                    ps[:, n * 512 : (n + 1) * 512], zl[:], zr[:],
                    start=True, stop=False,
                )

            # Ramp-up chunk sizes: small first chunks so PE gets data early
            # (the first gather sits behind the GpSimd library load), then
            # full CH-tile chunks (1024-idx dma_gather cap).
            chunks = []
            t_begin = 0
            for size in (1, 2):
                if t_begin < T:
                    n_t = min(size, T - t_begin)
                    chunks.append((t_begin, n_t))
                    t_begin += n_t
            while t_begin < T:
                n_t = min(CH, T - t_begin)
                chunks.append((t_begin, n_t))
                t_begin += n_t

            for c, (t_begin, n_t) in enumerate(chunks):
                gt = gpool.tile([128, CH, B], f16, tag="gt")
                n_idx = n_t * 128
                nc.gpsimd.dma_gather(
                    gt[:, 0:n_t, :],
                    xt_d.ap(),
                    idx_sb[:, t_begin * 8 : t_begin * 8 + n_t * 8],
                    num_idxs=n_idx,
                    num_idxs_reg=n_idx,
                    elem_size=B,
                    queue_num=c % NQ,
                )
                for tt in range(t_begin, t_begin + n_t):
                    lhsT = s_sb[:, tt * PC : (tt + 1) * PC]
                    for n in range(NB):
                        nc.tensor.matmul(
                            ps[:, n * 512 : (n + 1) * 512],
                            lhsT,
                            gt[:, tt - t_begin, n * 512 : (n + 1) * 512],
                            start=False,
                            stop=(tt == T - 1),
                        )

            # Per-bank eviction + store so bank n ships while bank n+1's last
            # matmul is still draining.
            for n in range(NB):
                ot = opool.tile([128, 512], f32, tag=f"ot{n}", name=f"ot{n}")
                nc.vector.tensor_scalar_mul(
                    ot[:], ps[:, n * 512 : (n + 1) * 512], inv_sb[:]
                )
                nc.sync.dma_start(out_d.ap()[:, n * 512 : (n + 1) * 512], ot[:])
    return nc


def _prepare(gene_set_features, flat_indices, segment_ids, segment_sizes):
    bounds, idx_sbs, s_sbs, T = _build_schedule(flat_indices, segment_ids)
    nc = _build_program(T)
    nc.compile()

    x = np.asarray(gene_set_features, dtype=np.float32)
    xt16 = np.ascontiguousarray(x.T.astype(np.float16))  # (G, B)
    sizes = np.asarray(segment_sizes, dtype=np.float32)

    in_maps = []
    for c in range(NCORES):
        lo_p, hi_p = bounds[c], bounds[c + 1]
        inv = np.ones((128, 1), np.float32)
        inv[: hi_p - lo_p, 0] = 1.0 / sizes[lo_p:hi_p]
        in_maps.append(
            {"xt": xt16, "idx": idx_sbs[c], "smat": s_sbs[c], "invsz": inv}
        )
    return nc, in_maps, bounds


def kernel(gene_set_features, flat_indices, segment_ids, segment_sizes, _res_hook=None):
    nc, in_maps, bounds = _prepare(
        gene_set_features, flat_indices, segment_ids, segment_sizes
    )
    res = run_bass_kernel_spmd(nc, in_maps, list(range(NCORES)))
    if _res_hook is not None:
        _res_hook(res)
    outT = np.empty((P, B), np.float32)
    for c in range(NCORES):
        lo_p, hi_p = bounds[c], bounds[c + 1]
        outT[lo_p:hi_p] = np.asarray(res.results[c]["out"])[: hi_p - lo_p]
    return np.ascontiguousarray(outT.T)


# revision 20
# speedup vs baseline: 1.6586x; 1.0726x over previous
"""Trainium2 Bass kernel for CellPathwayPoolingAggregator (segment mean).

out[b, p] = (1/segment_sizes[p]) * sum_{k: segment_ids[k]==p} x[b, flat_indices[k]]

Strategy (8 cores, sharded by contiguous pathway ranges):
  - Host: transpose x -> xT (G, B) fp16 (one shared copy per core). Split the
    1000 pathways into 8 contiguous ranges (<=128 pathways each) with roughly
    equal index counts.
  - Device (per core): gpsimd.dma_gather pulls the core's ~K/8 indexed
    gene-rows (full 2048-batch rows, 4KB each -> full DMA bandwidth) from
    DRAM into SBUF laid out [k%128, k//128, b]. For each 128-row K-tile a PE
    matmul with the per-core one-hot matrix S (128 k-rows x 128 local
    pathways, stationary) streams the gathered rows (N=512 x 4) and
    accumulates pathway x batch sums into one 4-bank PSUM tile.
  - DVE scales rows by 1/segment_sizes (per-partition scalar), DMA stores
    the (128, 2048) f32 transposed output slice; host reassembles/transposes.

All data-dependence lives in the per-core idx/S tensors, so the single SPMD
program is uniform across cores (T_max tiles each, zero-padded).
"""

import sys

import numpy as np

_TRN_REPO = "/opt/trn_rl_repo"
if _TRN_REPO not in sys.path:
    sys.path.insert(0, _TRN_REPO)

import concourse.bass as bass  # noqa: F401  (AP helpers via bass)
import concourse.mybir as mybir
import concourse.tile as tile
from concourse import bacc
from concourse.bass_utils import run_bass_kernel_spmd

B, G, P = 2048, 10000, 1000
NCORES = 8
PC = 128          # max pathways per core (psum partition dim)
NB = B // 512     # matmul N-slices per K-tile (4 banks of 512 f32)
CH = 4            # gather-chunk size in 128-index tiles (<=1024 idxs per dma_gather)
NQ = 4            # SWDGE queues, round-robin per chunk


def _split_ranges(seg_sorted):
    """Contiguous pathway ranges, <=128 pathways each, ~equal index counts."""
    K = len(seg_sorted)
    cnt = np.bincount(seg_sorted, minlength=P)
    cum = np.concatenate([[0], np.cumsum(cnt)])  # cum[p] = #entries below p
    bounds = [0]
    for c in range(1, NCORES):
        target = K * c // NCORES
        b = int(np.searchsorted(cum, target))
        b = max(bounds[-1] + 1, min(b, bounds[-1] + PC))
        b = max(b, P - PC * (NCORES - c))  # leave room for remaining cores
        b = min(b, P - (NCORES - c))       # leave >=1 pathway per core
        bounds.append(b)
    bounds.append(P)
    return bounds


def _build_schedule(flat_indices, segment_ids):
    seg = np.asarray(segment_ids, dtype=np.int64)
    idx = np.asarray(flat_indices, dtype=np.int64)
    order = np.argsort(seg, kind="stable")
    seg = seg[order]
    idx = idx[order]

    bounds = _split_ranges(seg)
    cores = []
    for c in range(NCORES):
        lo_p, hi_p = bounds[c], bounds[c + 1]
        lo = np.searchsorted(seg, lo_p, side="left")
        hi = np.searchsorted(seg, hi_p, side="left")
        # Deduplicate gene rows within the core: each distinct gene is
        # gathered once; S accumulates per-(gene,pathway) counts (exact in
        # fp16 for the counts seen here).
        uidx, inv = np.unique(idx[lo:hi], return_inverse=True)
        cores.append((lo_p, hi_p, uidx, inv, seg[lo:hi] - lo_p))

    T = max(1, max((len(u) + 127) // 128 for _, _, u, _, _ in cores))
    Kpad = T * 128

    idx_sbs, s_sbs = [], []
    for lo_p, hi_p, uidx, inv, cols in cores:
        nu = len(uidx)
        idx_p = np.concatenate([uidx, np.zeros(Kpad - nu, np.int64)])
        S = np.zeros((Kpad, PC), np.float32)
        np.add.at(S, (inv, cols), 1.0)
        S = S.astype(np.float16)
        s_sbs.append(
            np.ascontiguousarray(
                S.reshape(T, 128, PC).transpose(1, 0, 2).reshape(128, T * PC)
            )
        )
        idx16 = idx_p.astype(np.int16)
        idx_sbs.append(
            np.ascontiguousarray(np.tile(idx16.reshape(Kpad // 16, 16).T, (8, 1)))
        )
    return bounds, idx_sbs, s_sbs, T


def _build_program(T):
    nc = bacc.Bacc(
        "TRN2",
        target_bir_lowering=False,
        debug=False,
        num_devices=NCORES,
        num_swdge_queues=NQ,
    )
    f16, f32, i16 = mybir.dt.float16, mybir.dt.float32, mybir.dt.int16

    xt_d = nc.dram_tensor("xt", [G, B], f16, kind="ExternalInput")
    idx_d = nc.dram_tensor("idx", [128, T * 8], i16, kind="ExternalInput")
    s_d = nc.dram_tensor("smat", [128, T * PC], f16, kind="ExternalInput")
    inv_d = nc.dram_tensor("invsz", [128, 1], f32, kind="ExternalInput")
    out_d = nc.dram_tensor("out", [PC, B], f32, kind="ExternalOutput")

    with tile.TileContext(nc) as tc:
        with (
            tc.tile_pool(name="const", bufs=1) as cpool,
            tc.tile_pool(name="gather", bufs=6) as gpool,
            tc.tile_pool(name="psum", bufs=1, space="PSUM") as ppool,
            tc.tile_pool(name="outp", bufs=1) as opool,
        ):
            idx_sb = cpool.tile([128, T * 8], i16, tag="idx")
            nc.sync.dma_start(idx_sb[:], idx_d.ap())
            s_sb = cpool.tile([128, T * PC], f16, tag="smat")
            nc.sync.dma_start(s_sb[:], s_d.ap())
            inv_sb = cpool.tile([128, 1], f32, tag="invsz")
            nc.sync.dma_start(inv_sb[:], inv_d.ap())

            ps = ppool.tile([128, B], f32, tag="ps", name="ps")

            # Ramp-up chunk sizes: small first chunks so PE gets data early
            # (the first gather sits behind the GpSimd library load), then
            # full CH-tile chunks (1024-idx dma_gather cap).
            chunks = []
            t_begin = 0
            for size in (1, 2):
                if t_begin < T:
                    n_t = min(size, T - t_begin)
                    chunks.append((t_begin, n_t))
                    t_begin += n_t
            while t_begin < T:
                n_t = min(CH, T - t_begin)
                chunks.append((t_begin, n_t))
                t_begin += n_t

            for c, (t_begin, n_t) in enumerate(chunks):
                gt = gpool.tile([128, CH, B], f16, tag="gt")
                n_idx = n_t * 128
                nc.gpsimd.dma_gather(
                    gt[:, 0:n_t, :],
                    xt_d.ap(),
                    idx_sb[:, t_begin * 8 : t_begin * 8 + n_t * 8],
                    num_idxs=n_idx,
                    num_idxs_reg=n_idx,
                    elem_size=B,
                    queue_num=c % NQ,
                )
                for tt in range(t_begin, t_begin + n_t):
                    lhsT = s_sb[:, tt * PC : (tt + 1) * PC]
                    for n in range(NB):
                        nc.tensor.matmul(
                            ps[:, n * 512 : (n + 1) * 512],
                            lhsT,
                            gt[:, tt - t_begin, n * 512 : (n + 1) * 512],
                            # Every matmul writes the full (128, 512) bank
                            # region, so tile 0 with start=True both clears
                            # the bank's has_written bits and seeds the sums.
                            start=(tt == 0),
                            stop=(tt == T - 1),
                        )

            # Per-bank eviction + store so bank n ships while bank n+1's last
            # matmul is still draining.
            for n in range(NB):
                ot = opool.tile([128, 512], f32, tag=f"ot{n}", name=f"ot{n}")
                nc.vector.tensor_scalar_mul(
                    ot[:], ps[:, n * 512 : (n + 1) * 512], inv_sb[:]
                )
                nc.sync.dma_start(out_d.ap()[:, n * 512 : (n + 1) * 512], ot[:])
    return nc


def _prepare(gene_set_features, flat_indices, segment_ids, segment_sizes):
    bounds, idx_sbs, s_sbs, T = _build_schedule(flat_indices, segment_ids)
    nc = _build_program(T)
    nc.compile()

    x = np.asarray(gene_set_features, dtype=np.float32)
    xt16 = np.ascontiguousarray(x.T.astype(np.float16))  # (G, B)
    sizes = np.asarray(segment_sizes, dtype=np.float32)

    in_maps = []
    for c in range(NCORES):
        lo_p, hi_p = bounds[c], bounds[c + 1]
        inv = np.ones((128, 1), np.float32)
        inv[: hi_p - lo_p, 0] = 1.0 / sizes[lo_p:hi_p]
        in_maps.append(
            {"xt": xt16, "idx": idx_sbs[c], "smat": s_sbs[c], "invsz": inv}
        )
    return nc, in_maps, bounds


def kernel(gene_set_features, flat_indices, segment_ids, segment_sizes, _res_hook=None):
    nc, in_maps, bounds = _prepare(
        gene_set_features, flat_indices, segment_ids, segment_sizes
    )
    res = run_bass_kernel_spmd(nc, in_maps, list(range(NCORES)))
    if _res_hook is not None:
        _res_hook(res)
    outT = np.empty((P, B), np.float32)
    for c in range(NCORES):
        lo_p, hi_p = bounds[c], bounds[c + 1]
        outT[lo_p:hi_p] = np.asarray(res.results[c]["out"])[: hi_p - lo_p]
    return np.ascontiguousarray(outT.T)
